# revision 2
# baseline (speedup 1.0000x reference)
"""Multi-head attention Bass kernel v2 — 8 trn2 cores.

Sharding (as baseline): core c -> batch b=c//2, head-group g=c%2 (8 heads
= 4 pairs). Host combines: y[b] = part[2b] + part[2b+1] + bo + bv@Wo.

Per-core budget (@2.4GHz PE / 1.2GHz ACT): ACT exp of 33.5M scores in
2048-wide calls ~ 245-255us; PE QKV 196k + scores 262k (row-packed pairs)
+ attv 262k + oproj 66k cycles. All bf16 except psum/y (f32).

rsum_mode:
  "rider": attv M=65 (v | ones) serial per head; softmax denominators ride
           the attv matmul (PE-bound ~273us issued, DVE light).
  "dve":   attv M=64 col-packed pairs (concurrent); denominators via DVE
           bf16 accumulation + ones-matmul (ACT-bound, DVE ~240us).

PSUM banks: rider: BIG scores slot [128,2,2,512] (4) + o_ps x2 (2) +
fill x2 (2). dve: BIG (4) + MINI (2) + o_ps (1) + fill (1).
"""
import numpy as np

B, S, DIM, H, DH = 4, 2048, 1024, 16, 64
NCORES = 8
HPC = H // 2          # heads per core = 8
NPAIR = HPC // 2      # pairs = 4
NT = S // 128         # t-tiles = 16
NSC = S // 512        # s-chunks of 512 = 4
NKT = DIM // 128      # d-tiles = 8
SCALE = 1.0 / float(np.sqrt(DH))

_CACHE = {}


def _build(repeat=1, ncores=NCORES, rsum_mode="rider", debug_taps=False):
    import os
    ABL = os.environ.get("V2_ABLATE", "full")
    sk_scores = ABL in ("noscores", "attvonly")
    sk_attv = ABL in ("noattv", "scoresonly")
    sk_qkv = ABL in ("noqkv", "attonly")
    sk_oproj = ABL in ("nooproj", "attonly", "qkvonly")
    if ABL == "qkvonly":
        sk_scores = sk_attv = True
    import concourse.bass as bass
    import concourse.mybir as mybir
    import concourse.tile as tile
    from concourse import bacc
    from concourse.dve_ops import RECIP_APPROX_FAST_CONSTS

    f32 = mybir.dt.float32
    bf16 = mybir.dt.bfloat16
    AF = mybir.ActivationFunctionType
    Alu = mybir.AluOpType
    rider = rsum_mode == "rider"

    nc = bacc.Bacc("TRN2", target_bir_lowering=False, debug=False,
                   num_devices=ncores)

    # DRAM inputs (host-prepped, bf16)
    x_d = nc.dram_tensor("xT", [128, NKT, S], bf16, kind="ExternalInput")
    wq_d = nc.dram_tensor("wq", [128, NKT, NPAIR, 128], bf16,
                          kind="ExternalInput")
    wk_d = nc.dram_tensor("wk", [128, NKT, NPAIR, 128], bf16,
                          kind="ExternalInput")
    wv_d = nc.dram_tensor("wv", [128, NKT, HPC * DH], bf16,
                          kind="ExternalInput")
    wo_d = nc.dram_tensor("wo", [128, NPAIR, DIM], bf16, kind="ExternalInput")
    bq_d = nc.dram_tensor("bq", [1, NPAIR, 128], bf16,
                          kind="ExternalInput")
    bk_d = nc.dram_tensor("bk", [1, NPAIR, 128], bf16,
                          kind="ExternalInput")
    y_d = nc.dram_tensor("y", [S, DIM], f32, kind="ExternalOutput")

    VE = 65 if rider else 64   # v stationary cols per head
    if debug_taps:
        bf = mybir.dt.bfloat16
        dbg_qT = nc.dram_tensor("dbg_qT", [128, NPAIR, S], bf,
                                kind="ExternalOutput")
        dbg_kT = nc.dram_tensor("dbg_kT", [128, NPAIR, S], bf,
                                kind="ExternalOutput")
        dbg_vA = nc.dram_tensor("dbg_vA", [128, NT, HPC, VE], bf,
                                kind="ExternalOutput")
        dbg_on = nc.dram_tensor("dbg_on", [128, NPAIR, S], bf,
                                kind="ExternalOutput")
        dbg_att = nc.dram_tensor("dbg_att", [128, 2, 512], bf,
                                 kind="ExternalOutput")

    with tile.TileContext(nc) as tc:
        with tc.tile_pool(name="persist", bufs=1) as pp:
            xT = pp.tile([128, NKT, S], bf16)
            wqs = pp.tile([128, NKT, NPAIR, 128], bf16)
            wks = pp.tile([128, NKT, NPAIR, 128], bf16)
            wvs = pp.tile([128, NKT, HPC * DH], bf16)
            wos = pp.tile([128, NPAIR, DIM], bf16)
            bqs = pp.tile([1, NPAIR, 128], bf16)
            bks = pp.tile([1, NPAIR, 128], bf16)
            onesrow = pp.tile([1, 512], bf16)
            nc.vector.memset(onesrow, 1.0)
            onescol = pp.tile([128, 1], bf16)
            nc.vector.memset(onescol, 1.0)
            zeros128 = pp.tile([128, 128], bf16)
            nc.vector.memset(zeros128, 0.0)
            def load_inputs():
                for dst, src in ((xT, x_d), (wqs, wq_d), (wks, wk_d),
                                 (wvs, wv_d), (wos, wo_d), (bqs, bq_d),
                                 (bks, bk_d)):
                    nc.sync.dma_start(out=dst, in_=src.ap())

            qT = pp.tile([128, NPAIR, S], bf16)      # [2h*64e, p, s]
            kT = pp.tile([128, NPAIR, S], bf16)
            vA = pp.tile([128, NT, HPC, VE], bf16)   # [t%128, tt, h, e(|1)]
            if rider:
                nc.vector.memset(
                    vA.rearrange("p t h e -> p (t h) e")[:, :, 64:65], 1.0)
            onorm = pp.tile([128, NPAIR, S], bf16)   # normalized o^T
            if sk_attv:
                nc.vector.memset(onorm, 0.01)
            if sk_qkv:
                nc.vector.memset(qT, 0.02)
                nc.vector.memset(kT, 0.02)
                nc.vector.memset(vA, 0.02)
            attfix = None
            if sk_scores:
                attfix = pp.tile([128, 2, 2, 512], bf16)
                nc.vector.memset(attfix, 0.5)

            for rep in range(repeat):
              load_inputs()
              with tc.tile_pool(name=f"w{rep}", bufs=1) as wp, \
                   tc.tile_pool(name=f"ps{rep}", bufs=1, space="PSUM") as psp:

                FILLB = 2 if rider else 1

                # ---------- filler emitters ----------
                def emit_qk(proj, p, sc):
                    if sk_qkv:
                        return
                    wsrc, dst, bias = ((wqs, qT, bqs) if proj == "q"
                                       else (wks, kT, bks))
                    fps = psp.tile([128, 512], f32, tag="fill", bufs=FILLB,
                                   name=f"f{proj}{p}{sc}_{rep}")
                    for kt in range(NKT):
                        nc.tensor.matmul(
                            fps, wsrc[:, kt, p, :],
                            xT[:, kt, sc * 512:(sc + 1) * 512],
                            start=(kt == 0), stop=False)
                    nc.tensor.matmul(
                        fps, bias[0:1, p, :], onesrow,
                        start=False, stop=True)
                    nc.vector.tensor_copy(
                        dst[:, p, sc * 512:(sc + 1) * 512], fps)

                def emit_v(ttg):
                    if sk_qkv:
                        return
                    fps = psp.tile([128, 512], f32, tag="fill", bufs=FILLB,
                                   name=f"fv{ttg}_{rep}")
                    for kt in range(NKT):
                        nc.tensor.matmul(
                            fps, xT[:, kt, ttg * 128:(ttg + 1) * 128],
                            wvs[:, kt, :],
                            start=(kt == 0), stop=(kt == NKT - 1))
                    nc.vector.tensor_copy(
                        vA[:, ttg, :, 0:64],
                        fps.rearrange("p (h e) -> p h e", h=HPC))

                def emit_oproj(sc):
                    if sk_oproj:
                        return
                    for st2 in range(4):
                        st = sc * 4 + st2
                        ysb = wp.tile([128, DIM], f32, tag="ysb", bufs=2,
                                      name=f"y{st}_{rep}")
                        for nh in range(2):
                            fps = psp.tile([128, 512], f32, tag="fill",
                                           bufs=FILLB,
                                           name=f"fy{st}{nh}_{rep}")
                            for p in range(NPAIR):
                                nc.tensor.matmul(
                                    fps,
                                    onorm[:, p, st * 128:(st + 1) * 128],
                                    wos[:, p, nh * 512:(nh + 1) * 512],
                                    start=(p == 0), stop=(p == NPAIR - 1))
                            nc.vector.tensor_copy(
                                ysb[:, nh * 512:(nh + 1) * 512], fps)
                        nc.sync.dma_start(
                            out=y_d.ap()[st * 128:(st + 1) * 128, :],
                            in_=ysb)

                # ---------- attention for (p, sc) ----------
                def emit_attention(p, sc, filler, pre_slot=None):
                    """filler: thunks emitted after slots (PE slack work,
                    consumed only by later chunks). pre_slot: dict
                    slot_idx -> [thunks] emitted BEFORE that slot (producers
                    of data this chunk's slots read)."""
                    pre_slot = pre_slot or {}
                    sqs = slice(sc * 512, (sc + 1) * 512)
                    if rider:
                        o_ps = [psp.tile([65, 512], f32, tag=f"ops{h}",
                                         bufs=1, name=f"o{p}{sc}{h}_{rep}")
                                for h in range(2)]
                    else:
                        o_ps = psp.tile([128, 512], f32, tag="ops", bufs=1,
                                        name=f"o{p}{sc}_{rep}")
                        # zero-init the bank (sets has_written everywhere) so
                        # both col-packed heads can accumulate with flags=0 —
                        # avoids the bank-wide-clear race between the pair.
                        nc.tensor.matmul(o_ps, zeros128,
                                         xT[:, 0, 0:512],
                                         start=True, stop=False,
                                         skip_group_check=True)
                        acc = wp.tile([128, 2, 512], bf16, tag="acc", bufs=2,
                                      name=f"acc{p}{sc}_{rep}")
                    fi = 0
                    if rider:
                        # width-1024 slots, double-buffered (no exp->scores
                        # serialization; ACT has slack in PE-bound rider mode)
                        slots = [("mini", [tt]) for tt in range(NT)]
                    else:
                        slots = []
                        tt = 0
                        for i in range(5):
                            slots.append(("big", [tt, tt + 1])); tt += 2
                            slots.append(("mini", [tt])); tt += 1
                        slots.append(("mini", [tt]))
                    for si, (kind, tts) in enumerate(slots):
                        for th in pre_slot.get(si, ()):
                            th()
                        n = len(tts)
                        if sk_scores:
                            att = attfix
                        else:
                            sps = psp.tile([128, 2, n, 512], f32, tag=kind,
                                           bufs=(2 if rider else 1),
                                           name=f"s{p}{sc}{si}_{rep}")
                            att = wp.tile([128, 2, n, 512], bf16,
                                          tag=f"att{kind}", bufs=2,
                                          name=f"a{p}{sc}{si}_{rep}")
                            for j, tt_ in enumerate(tts):
                                for h in range(2):
                                    lo = h * 64
                                    nc.tensor.matmul(
                                        sps[:, h, j, :],
                                        kT[lo:lo + 64, p,
                                           tt_ * 128:(tt_ + 1) * 128],
                                        qT[lo:lo + 64, p, sqs],
                                        start=True, stop=True)
                            if n == 1:
                                nc.scalar.activation(att[:, :, 0, :],
                                                     sps[:, :, 0, :],
                                                     AF.Exp, scale=SCALE)
                            else:
                                nc.scalar.activation(att, sps, AF.Exp,
                                                     scale=SCALE)
                        if sk_attv:
                            if fi < len(filler):
                                filler[fi](); fi += 1
                            continue
                        if debug_taps and p == 0 and sc == 0 and si == 0:
                            nc.sync.dma_start(
                                out=dbg_att.ap(),
                                in_=att[:, :, 0, :])
                        for j, tt_ in enumerate(tts):
                            for h in range(2):
                                if rider:
                                    nc.tensor.matmul(
                                        o_ps[h],
                                        vA[:, tt_, 2 * p + h, :],
                                        att[:, h, j, :],
                                        start=(tt_ == 0),
                                        stop=(tt_ == NT - 1))
                                else:
                                    # col-packed pair shares one psum bank;
                                    # start=True clears has_written BANK-wide,
                                    # so only h0's first matmul starts the
                                    # group — h1's first (flags=0, bits
                                    # cleared) overwrites its region.
                                    lo = h * 64
                                    nc.tensor.matmul(
                                        o_ps[lo:lo + 64, :],
                                        vA[:, tt_, 2 * p + h, :],
                                        att[:, h, j, :],
                                        start=False,
                                        stop=(tt_ == NT - 1 and h == 1),
                                        tile_position=(0, lo),
                                        skip_group_check=True)
                        if not rider:
                            for h in range(2):
                                if tts[0] == 0:
                                    nc.vector.tensor_copy(
                                        acc[:, h, :], att[:, h, 0, :])
                                    for j in range(1, n):
                                        nc.vector.tensor_tensor(
                                            out=acc[:, h, :],
                                            in0=acc[:, h, :],
                                            in1=att[:, h, j, :], op=Alu.add)
                                else:
                                    for j in range(n):
                                        nc.vector.tensor_tensor(
                                            out=acc[:, h, :],
                                            in0=acc[:, h, :],
                                            in1=att[:, h, j, :], op=Alu.add)
                        if fi < len(filler):
                            filler[fi](); fi += 1
                    while fi < len(filler):
                        filler[fi](); fi += 1

                    # ---------- normalize ----------
                    if sk_attv:
                        return
                    if rider:
                        rsrc = [o_ps[h][64:65, :] for h in range(2)]
                    else:
                        rps = psp.tile([128, 512], f32, tag="fill",
                                       bufs=FILLB, name=f"r{p}{sc}_{rep}")
                        nc.tensor.matmul(rps[0:1, :], onescol, acc[:, 0, :],
                                         start=True, stop=True,
                                         tile_position=(0, 0))
                        nc.tensor.matmul(rps[32:33, :], onescol,
                                         acc[:, 1, :],
                                         start=True, stop=True,
                                         tile_position=(0, 32))
                        rsrc = [rps[0:1, :], rps[32:33, :]]
                    rinv = [wp.tile([1, 512], f32, tag=f"rinv{h}", bufs=2,
                                    name=f"ri{p}{sc}{h}_{rep}")
                            for h in range(2)]
                    rrow = [wp.tile([1, 512], f32, tag=f"rrow{h}", bufs=2,
                                    name=f"rr{p}{sc}{h}_{rep}")
                            for h in range(2)]
                    for h in range(2):
                        # stage to base-0 SBUF first: custom-DVE ops misread
                        # partition-offset PSUM rows
                        nc.vector.tensor_copy(rrow[h], rsrc[h])
                        nc.vector.reciprocal_approx_fast(rinv[h], rrow[h])
                    # all partition-base-0 tiles; partition placement into
                    # onorm[64:128] only via tensor_copy (as baseline; direct
                    # base-shifted DVE/gpsimd ops corrupt SBUF on HW)
                    rbc = [wp.tile([64, 512], f32, tag=f"rbc{h}", bufs=2,
                                   name=f"rb{p}{sc}{h}_{rep}")
                           for h in range(2)]
                    for h in range(2):
                        nc.gpsimd.partition_broadcast(rbc[h], rinv[h])
                    for h in range(2):
                        src_o = (o_ps[h][0:64, :] if rider
                                 else o_ps[h * 64:(h + 1) * 64, :])
                        if h == 0:
                            nc.vector.tensor_tensor(
                                out=onorm[0:64, p, sqs],
                                in0=src_o, in1=rbc[0], op=Alu.mult)
                        else:
                            tmp1 = wp.tile([64, 512], bf16, tag="ntmp",
                                           bufs=2, name=f"nt{p}{sc}_{rep}")
                            nc.vector.tensor_tensor(
                                out=tmp1, in0=src_o, in1=rbc[1],
                                op=Alu.mult)
                            nc.vector.tensor_copy(
                                onorm[64:128, p, sqs], tmp1)

                # ---------- emission schedule ----------
                # scores(p, sc, slot tt) reads kT[p, tt*128...] (k-chunk
                # tt//4) and qT[p, sc]; attv reads vA[tt]. Producers must be
                # EMITTED before their consuming slot (program order defines
                # dataflow). Chunk order: sc outer, p inner; oproj(sc) after
                # all pairs of sc.
                emit_qk("k", 0, 0)
                emit_qk("q", 0, 0)

                for sc in range(NSC):
                    for p in range(NPAIR):
                        pre = {}
                        fill = []
                        if sc == 0 and p == 0:
                            # v tiles + own later k-chunks, JIT per slot
                            if rider:   # slot si covers tt=si
                                for si in range(NT):
                                    pre.setdefault(si, [])
                                    if si % 4 == 0 and si > 0:
                                        pre[si].append(
                                            lambda sck=si // 4:
                                            emit_qk("k", 0, sck))
                                    pre[si].append(
                                        lambda tt=si: emit_v(tt))
                            else:       # slot->tts mapping as emit_attention
                                spans = []
                                tt = 0
                                for i in range(5):
                                    spans.append([tt, tt + 1]); tt += 2
                                    spans.append([tt]); tt += 1
                                spans.append([tt])
                                for si, tts in enumerate(spans):
                                    pre.setdefault(si, [])
                                    for tt_ in tts:
                                        if tt_ % 4 == 0 and tt_ > 0:
                                            pre[si].append(
                                                lambda sck=tt_ // 4:
                                                emit_qk("k", 0, sck))
                                        pre[si].append(
                                            lambda tt=tt_: emit_v(tt))
                        if sc == 0 and p < NPAIR - 1:
                            # next pair's k (all chunks) + q(sc0)
                            for sck in range(NSC):
                                fill.append(
                                    lambda p2=p + 1, sck=sck:
                                    emit_qk("k", p2, sck))
                            fill.append(
                                lambda p2=p + 1: emit_qk("q", p2, 0))
                        if sc + 1 < NSC:
                            if sc == 0 and p == NPAIR - 1:
                                for p2 in range(NPAIR):
                                    fill.append(
                                        lambda p2=p2: emit_qk("q", p2, 1))
                            elif sc > 0:
                                fill.append(
                                    lambda p2=p, sc2=sc + 1:
                                    emit_qk("q", p2, sc2))
                        emit_attention(p, sc, fill, pre)
                    emit_oproj(sc)
                if debug_taps:
                    nc.sync.dma_start(out=dbg_qT.ap(), in_=qT)
                    nc.sync.dma_start(out=dbg_kT.ap(), in_=kT)
                    nc.sync.dma_start(out=dbg_vA.ap(), in_=vA)
                    nc.sync.dma_start(out=dbg_on.ap(), in_=onorm)
    nc.compile()
    return nc


def _get_nc():
    if "nc" not in _CACHE:
        _CACHE["nc"] = _build()
    return _CACHE["nc"]


def make_in_maps(x, Wq, Wk, Wv, bq, bk, bv, Wo, bo):
    import ml_dtypes
    bf16 = ml_dtypes.bfloat16
    in_maps = []
    for c in range(NCORES):
        b, g = c // 2, c % 2
        hs = slice(g * HPC, (g + 1) * HPC)
        xt = np.ascontiguousarray(
            x[b].T.reshape(NKT, 128, S).transpose(1, 0, 2)).astype(bf16)

        def qk(W):
            w = W[hs]                                # [8, 1024, 64]
            w = w.reshape(NPAIR, 2, DIM, DH)         # [p, 2h, d, e]
            w = w.transpose(2, 0, 1, 3).reshape(DIM, NPAIR, 128)
            w = w.reshape(NKT, 128, NPAIR, 128).transpose(1, 0, 2, 3)
            return np.ascontiguousarray(w).astype(bf16)
        wv = Wv[hs].transpose(1, 0, 2).reshape(DIM, HPC * DH)
        wv = wv.reshape(NKT, 128, HPC * DH).transpose(1, 0, 2)
        wo = Wo[g * 512:(g + 1) * 512, :].reshape(NPAIR, 128, DIM)
        wo = wo.transpose(1, 0, 2)
        in_maps.append({
            "xT": xt,
            "wq": qk(Wq), "wk": qk(Wk),
            "wv": np.ascontiguousarray(wv).astype(bf16),
            "wo": np.ascontiguousarray(wo).astype(bf16),
            "bq": np.ascontiguousarray(
                bq[hs].reshape(1, NPAIR, 128)).astype(bf16),
            "bk": np.ascontiguousarray(
                bk[hs].reshape(1, NPAIR, 128)).astype(bf16),
        })
    return in_maps


def combine(results, bv, Wo, bo):
    const = bv.reshape(DIM) @ Wo + bo
    y = np.empty((B, S, DIM), dtype=np.float32)
    for b in range(B):
        y[b] = results[2 * b]["y"] + results[2 * b + 1]["y"] + const
    return y


def kernel(x, Wq, Wk, Wv, bq, bk, bv, Wo, bo):
    import time
    from concourse.bass_utils import run_bass_kernel_spmd
    x, Wq, Wk, Wv, bq, bk, bv, Wo, bo = [
        np.asarray(a, dtype=np.float32)
        for a in (x, Wq, Wk, Wv, bq, bk, bv, Wo, bo)]
    nc = _get_nc()
    in_maps = make_in_maps(x, Wq, Wk, Wv, bq, bk, bv, Wo, bo)
    last = None
    for attempt in range(3):
        try:
            res = run_bass_kernel_spmd(nc, in_maps,
                                       core_ids=list(range(NCORES)))
            return combine(res.results, bv, Wo, bo)
        except Exception as e:
            last = e
            time.sleep(75)
    raise last
